# revision 54
# baseline (speedup 1.0000x reference)
"""BERT layer (B=8, S=512, H=768, NH=12, DH=64, FF=3072) on 8 Trainium2 cores.

Data-parallel over batch (1 element/core).  Feature-major on-chip layout
(activations as X^T [H partitions, S free]).  All contraction>=256 matmuls run
fp8e4m3 DoubleRow (2x PE rate): QKV projections, ctx, Wo, FFN1.  FFN2 stays
bf16 for accuracy.  Weights are host-scaled x64 so fp8 stays in normal range;
the 1/64 unscale is folded into the PSUM-evacuation ops that exist anyway.
Scores stay bf16 (K=64, row-packed head pairs); LayerNorm stats stay f32r.

Softmax: the additive 0/1 mask is folded multiplicatively (exp(s+mb)=exp(s)*m):
V is scaled by m/64 on evacuation, and the softmax denominator comes FOR FREE
out of the ctx matmul: the ctx stationary operand is widened to 128 columns
[V_head | mask/64 replicated], so PSUM rows 0-63 hold the ctx numerator and
rows 64-127 hold the denominator (replicated).  recip = 64/den cancels the
1/64 and lands ctx at x64 scale, the fp8-friendly range for the Wo input.

LayerNorm inputs are pre-shifted so no bias plumbing is needed in the stats:
  LN1: xT carries x^T + bo_eff (bo_eff = bo + bv@Wo) from the host.
  LN2: r2 = ffn2 + c2 + q32 in one scalar_tensor_tensor (c2 = b2 + ln1_b).
rstd = exp(-0.5*ln(var+eps)) keeps the ACT table set at
natural_log_exp_and_others (shared with softmax exp): only 2 table switches
per layer (to/from the Gelu set).

Engine balance: ACT keeps only table ops (exp/gelu/ln-exp); squares, copies,
and the final scale+bias run on DVE; cen/mean^2/q32/nrm run on Pool (gpsimd),
which is otherwise idle.  FFN weight DMAs issue from the sync queue.

ln1_b (beta1) folding (exact): h1 = gamma1*nrm + beta1.  The fp8 h1 fed to
FFN1 omits beta1 (compensated by b1_eff = b1 + beta1@W1); the residual
carries q = gamma1*nrm exactly, with beta1 folded into LN2's shift
c2 = b2 + beta1.
"""

from contextlib import ExitStack

import numpy as np
import ml_dtypes

from concourse import bacc
import concourse.tile as tile
from concourse import mybir
from concourse.bass_utils import run_bass_kernel_spmd

F32 = mybir.dt.float32
F32R = mybir.dt.float32r
BF16 = mybir.dt.bfloat16
F8 = mybir.dt.float8e4
AF = mybir.ActivationFunctionType
ALU = mybir.AluOpType
PM = mybir.MatmulPerfMode

B, S, H, NH, DH, FF = 8, 512, 768, 12, 64, 3072
CH = H // 128   # 6 hidden chunks
CF = FF // 128  # 24 ff chunks
T = S // 128    # 4 token/key chunks
NP = NH // 2    # 6 head pairs
EPS = 1e-3
FFN_MODE = "mix"  # "mix" (ff1 fp8 + ff2 bf16) | "bf16"
SW = 64.0       # weight scale (keeps fp8 in normal range)
ISW = 1.0 / SW

# consts tile column map: [128, NCONST]
BQ, BK, L1G, C2, L2G, L2B = 0, 6, 12, 18, 24, 30
MCOL = 36        # 4 cols: mask/64 per key chunk
B1E = 40         # 24 cols: b1 + ln1_b @ W1
NCONST = B1E + CF


def ts(i, n):
    return slice(i * n, (i + 1) * n)


def build_nc(repeats=1, ffn_mode=None, dbg=False, upto="full"):
    ffn_mode = ffn_mode or FFN_MODE
    nc = bacc.Bacc("TRN2", target_bir_lowering=False, debug=False)
    dbg_d = {}
    if dbg:
        for nm, shp, dt_ in (
                ("d_qT", [128, CH, S], BF16), ("d_kT", [128, CH, S], BF16),
                ("d_v8m", [128, T, NP, 2, 128], F8), ("d_es0", [128, T, S], F8),
                ("d_ctx8", [128, CH, S], F8), ("d_r1", [128, CH, S], F32R),
                ("d_q32", [128, CH, S], F32), ("d_q8", [128, CH, S], F8),
                ("d_gel", [128, CF, S], BF16), ("d_r2", [128, CH, S], F32R)):
            dbg_d[nm] = nc.declare_dram_parameter(nm, shp, dt_, isOutput=True)

    def dump(nm, t):
        if dbg:
            nc.sync.dma_start(out=dbg_d[nm][...], in_=t)

    xT_d = nc.declare_dram_parameter("xT", [H, S], F32R, isOutput=False)
    xT8_d = nc.declare_dram_parameter("xT8", [H, S], F8, isOutput=False)
    wqk_d = nc.declare_dram_parameter("wqkb", [CH, 128, 2, CH, 128], F8,
                                      isOutput=False)
    wv_d = nc.declare_dram_parameter("wv", [H, H], F8, isOutput=False)
    wob_d = nc.declare_dram_parameter("wob", [CH, 128, CH, 128], F8,
                                      isOutput=False)
    W1DT = F8 if ffn_mode == "mix" else BF16
    w1_d = nc.declare_dram_parameter("w1b", [CF, 128, CH, 128], W1DT,
                                     isOutput=False)
    w2_d = nc.declare_dram_parameter("w2b", [CH, 128, CF, 128], BF16,
                                     isOutput=False)
    c_d = nc.declare_dram_parameter("consts", [128, NCONST], F32,
                                    isOutput=False)
    out_d = nc.declare_dram_parameter("outT", [H, S], F32, isOutput=True)

    def fmaj(d):
        return d.rearrange("(i p) n -> p i n", p=128)

    with tile.TileContext(nc) as tc, ExitStack() as top:
        cpool = top.enter_context(tc.tile_pool(name="cpool", bufs=1))
        c_sb = cpool.tile([128, NCONST], F32, name="c_sb")
        nc.sync.dma_start(out=c_sb, in_=c_d[:, :])
        ones_f32 = cpool.tile([128, 384], F32, name="ones_f32")
        nc.vector.memset(ones_f32, 1.0)
        ones_sum = cpool.tile([128, 128], F32R, name="ones_sum")
        nc.vector.tensor_copy(out=ones_sum, in_=ones_f32[:, 0:128])

        mid = top.enter_context(tc.tile_pool(name="mid", bufs=1))
        tmp = top.enter_context(tc.tile_pool(name="tmp", bufs=1))
        fpool = top.enter_context(tc.tile_pool(name="fpool", bufs=1))
        w1pool = top.enter_context(tc.tile_pool(name="w1p", bufs=3))
        w2pool = top.enter_context(tc.tile_pool(name="w2p", bufs=2))

        class LN:
            """Incremental LayerNorm over pre-shifted feature-major src.

            accum(i) is called as each src chunk becomes ready so the
            sum/sq matmuls interleave with the producing loop; the sq
            matmuls trail two chunks so their DVE/Pool square op has time
            to complete before the PE reaches the matmul.
            """

            def __init__(self, pssum):
                self.sum_ps = pssum.tile([128, S], F32, tag="lnsum", bufs=1,
                                         name="sum_ps")
                self.sq_ps = pssum.tile([128, S], F32, tag="lnsq", bufs=1,
                                        name="sq_ps")
                self.pending = []
                self.k = 0

            def accum(self, src_i, i):
                nc.tensor.matmul(self.sum_ps[:, :], ones_sum[:, :], src_i,
                                 start=(i == 0), stop=(i == CH - 1))
                # Square on ACT: in every table set (no table-load cost) and
                # ACT is idle in both LN windows
                sq = tmp.tile([128, S], F32R, tag="sq", bufs=3, name="sq")
                nc.scalar.activation(out=sq, in_=src_i, func=AF.Square)
                self.pending.append(sq)
                while len(self.pending) > 2:
                    self._sq_mm()

            def _sq_mm(self):
                sq = self.pending.pop(0)
                nc.tensor.matmul(self.sq_ps[:, :], ones_sum[:, :], sq,
                                 start=(self.k == 0), stop=(self.k == CH - 1))
                self.k += 1

            def finish_stats(self):
                """Evacuate the PSUM sums (the only PSUM-dependent step)."""
                while self.pending:
                    self._sq_mm()
                self.mean = tmp.tile([128, S], F32, tag="mean", bufs=2,
                                     name="mean")
                nc.scalar.activation(out=self.mean, in_=self.sum_ps[:, :],
                                     func=AF.Identity, scale=1.0 / H)
                self.ve = tmp.tile([128, S], F32, tag="ve", bufs=2,
                                   name="ve")
                nc.vector.tensor_scalar(
                    out=self.ve, in0=self.sq_ps[:, :], scalar1=1.0 / H,
                    scalar2=EPS, op0=ALU.mult, op1=ALU.add)

            def defer_stats(self):
                """SBUF-only stats chain; safe to run any time after
                finish_stats, e.g. overlapped with the next rep."""
                m2 = tmp.tile([128, S], F32, tag="m2", bufs=2, name="m2")
                nc.gpsimd.tensor_tensor(out=m2, in0=self.mean, in1=self.mean,
                                        op=ALU.mult)
                var = tmp.tile([128, S], F32, tag="var", bufs=2, name="var")
                nc.vector.tensor_tensor(out=var, in0=self.ve, in1=m2,
                                        op=ALU.subtract)
                std = tmp.tile([128, S], F32, tag="std", bufs=2, name="std")
                nc.scalar.activation(out=std, in_=var, func=AF.Sqrt)
                self.rstd = tmp.tile([128, S], F32, tag="rstd", bufs=2,
                                     name="rstd")
                nc.vector.reciprocal_approx_fast(out=self.rstd, in_=std)

            def emit_chunk(self, src, jj, emit, spread=False):
                cen = tmp.tile([128, S], F32, tag="cen", bufs=2,
                               name="cen")
                eng = nc.gpsimd if (spread and jj % 2 == 0) else nc.vector
                eng.tensor_tensor(out=cen, in0=src[:, jj, :], in1=self.mean,
                                  op=ALU.subtract)
                emit(jj, cen, self.rstd)

            def finish(self, src, emit):
                self.finish_stats()
                self.defer_stats()
                for jj in range(CH):
                    self.emit_chunk(src, jj, emit)

        # deferred LN2 of the previous rep: SBUF-only ops overlapped with the
        # next rep's attention phase (which has PE/exp as its bottleneck)
        prev_fin = None
        prev_emit = None

        for _rep in range(repeats):
            with ExitStack() as s_ac:
                apool = s_ac.enter_context(tc.tile_pool(name="apool", bufs=1))
                xT = apool.tile([128, CH, S], F32R, tag="xT", bufs=1, name="xT")
                xT8 = apool.tile([128, CH, S], F8, tag="xT8", bufs=1, name="xT8")
                nc.sync.dma_start(out=xT8, in_=fmaj(xT8_d))
                # xT (residual) is first read at the Wo evacuation; issue it
                # on the idle Pool queue (SWDGE) so it doesn't delay the
                # early-phase sync-queue inputs
                nc.gpsimd.dma_start(out=xT, in_=fmaj(xT_d))
                qT = apool.tile([128, CH, S], BF16, tag="qT", bufs=1, name="qT")
                kT = apool.tile([128, CH, S], BF16, tag="kT", bufs=1, name="kT")
                # v8m [p, t, pair, half, 128]: per head 128 stationary cols.
                # Even heads (half 0): [mask/64 | V]; odd heads: [V | mask/64].
                # The parity-dependent layout keeps the reciprocal's in/out
                # partition bases equal (custom-DVE op requires it).
                v8m = apool.tile([128, T, NP, 2, 128], F8, tag="v8m", bufs=2,
                                 name="v8m")
                ctx8 = apool.tile([128, CH, S], F8, tag="ctx8", bufs=1, name="ctx8")

                wpool = s_ac.enter_context(tc.tile_pool(name="wpool", bufs=1))
                wo_sb = wpool.tile([128, CH, CH, 128], F8, name="wo_sb")

                psP = s_ac.enter_context(
                    tc.tile_pool(name="psP", bufs=1, space="PSUM"))
                wqkpool = s_ac.enter_context(tc.tile_pool(name="wqk", bufs=3))

                def project_qk(j):
                    wt = wqkpool.tile([128, 2, CH, 128], F8, tag="wqk",
                                      name="wt")
                    nc.sync.dma_start(out=wt, in_=wqk_d[j])
                    for ci, (dest, bcol) in enumerate(((qT, BQ), (kT, BK))):
                        ps = psP.tile([128, S], F32, tag="pj", bufs=1,
                                      name="ps_qk")
                        for i in range(0, CH, 2):
                            nc.tensor.matmul(
                                ps[:, :], wt[:, ci, i:i + 2, :],
                                xT8[:, i:i + 2, :],
                                start=(i == 0), stop=(i == CH - 2),
                                perf_mode=PM.DoubleRow)
                        nc.vector.tensor_scalar(
                            out=dest[:, j, :], in0=ps[:, :], scalar1=ISW,
                            scalar2=c_sb[:, bcol + j:bcol + j + 1],
                            op0=ALU.mult, op1=ALU.add)

                # mask/64 replica columns of v8m (per key chunk t)
                ones6 = ones_f32[:, 0:384].rearrange("p (h d) -> p h d", h=NP)
                for t in range(T):
                    for half in range(2):
                        nc.vector.tensor_scalar(
                            out=v8m[:, t, :, half, ts(half, 64)],
                            in0=ones6,
                            scalar1=c_sb[:, MCOL + t:MCOL + t + 1],
                            scalar2=None, op0=ALU.mult)

                # ---- V projection (token-major), then Q/K of pair 0 ----
                with ExitStack() as s_v:
                    wvpool = s_v.enter_context(
                        tc.tile_pool(name="wvp", bufs=1))
                    wv_sb = wvpool.tile([128, CH, H], F8, name="wv_sb")
                    nc.sync.dma_start(out=wv_sb, in_=fmaj(wv_d))
                    psV = s_v.enter_context(
                        tc.tile_pool(name="psV", bufs=1, space="PSUM"))
                    for t in range(T):
                        for half in range(2):
                            ps = psV.tile([128, 512], F32, tag="mv", bufs=4,
                                          name="ps_v")
                            for i in range(0, CH, 2):
                                nc.tensor.matmul(
                                    ps[:, 0:384],
                                    xT8[:, i:i + 2, ts(t, 128)],
                                    wv_sb[:, i:i + 2, ts(half, 384)],
                                    start=(i == 0), stop=(i == CH - 2),
                                    perf_mode=PM.DoubleRow)
                            # wv cols are host-permuted by head parity:
                            # vhalf 0 = even heads, vhalf 1 = odd heads
                            nc.vector.tensor_scalar(
                                out=v8m[:, t, :, half, ts(1 - half, 64)],
                                in0=ps[:, 0:384].rearrange(
                                    "p (h d) -> p h d", h=6),
                                scalar1=c_sb[:, MCOL + t:MCOL + t + 1],
                                scalar2=None, op0=ALU.mult)
                    project_qk(0)

                # ---- pair loop: scores -> exp -> ctx+den ----
                with ExitStack() as s_b:
                    psS = s_b.enter_context(
                        tc.tile_pool(name="psS", bufs=1, space="PSUM"))
                    psD = s_b.enter_context(
                        tc.tile_pool(name="psD", bufs=1, space="PSUM"))
                    bpool = s_b.enter_context(
                        tc.tile_pool(name="bpool", bufs=1))
                    es_store = {}

                    def do_scores(j):
                        es_ab = []
                        for half in range(2):
                            es = bpool.tile([128, T, S], F8, tag=f"es{half}",
                                            bufs=3, name=f"es{half}")
                            es_ab.append(es)
                            for u in range(2):
                                sc_ps = psS.tile([128, 1024], F32, tag="sc",
                                                 bufs=2, name="sc_ps")
                                for v_ in range(2):
                                    t = 2 * u + v_
                                    nc.tensor.matmul(
                                        sc_ps[:, ts(v_, 512)],
                                        kT[ts(half, 64), j, ts(t, 128)],
                                        qT[ts(half, 64), j, :],
                                        start=True, stop=True,
                                        tile_position=(half * 64, 0))
                                nc.scalar.activation(
                                    out=es[:, 2 * u:2 * u + 2, :].rearrange(
                                        "p a b -> p (a b)"),
                                    in_=sc_ps[:, :], func=AF.Exp, scale=0.125)
                        es_store[j] = es_ab

                    def do_ctx(j):
                        # ctx+den: DoubleRow per head; stationary 128 cols =
                        # [mask/64 | V] (even heads) / [V | mask/64] (odd), so
                        # one PSUM half is the numerator and the other the
                        # replicated denominator.  The reciprocal (custom-DVE
                        # microcode) needs full base-0 tiles, so it runs over
                        # all 128 rows; non-den rows are garbage, never read.
                        # The parity keeps the multiply's SBUF in1/out
                        # partition bases equal.
                        es_ab = es_store.pop(j)
                        for half, es in enumerate(es_ab):
                            ctx_ps = psD.tile([128, S], F32, tag="ctx",
                                              bufs=2, name="ctx_ps")
                            for u in range(2):
                                nc.tensor.matmul(
                                    ctx_ps[:, :],
                                    v8m[:, 2 * u:2 * u + 2, j, half, :],
                                    es[:, 2 * u:2 * u + 2, :],
                                    start=(u == 0), stop=(u == 1),
                                    perf_mode=PM.DoubleRow)
                            recip = bpool.tile([128, S], F32, tag="recip",
                                               bufs=2, name="recip")
                            nc.vector.reciprocal_approx_fast(
                                out=recip, in_=ctx_ps[:, :])
                            nc.vector.tensor_tensor(
                                out=ctx8[ts(half, 64), j, :],
                                in0=ctx_ps[ts(1 - half, 64), :],
                                in1=recip[ts(half, 64), :], op=ALU.mult)

                    # software pipeline: ctx runs one pair behind scores so
                    # the in-order PE queue never waits on the exp (ACT) of
                    # the current pair.
                    for j in range(NP):
                        if j > 0:
                            project_qk(j)
                        if j == 2:
                            nc.sync.dma_start(
                                out=wo_sb,
                                in_=wob_d.rearrange("j p i m -> p j i m"))
                        do_scores(j)
                        if j > 0:
                            do_ctx(j - 1)
                    do_ctx(NP - 1)

                dump("d_qT", qT)
                dump("d_kT", kT)
                dump("d_v8m", v8m)
                dump("d_ctx8", ctx8)

                if upto == "attn":
                    for jj in range(CH):
                        ct = tmp.tile([128, S], F32, tag="ct", bufs=2,
                                      name="ct")
                        nc.vector.tensor_copy(out=ct, in_=ctx8[:, jj, :])
                        nc.sync.dma_start(out=out_d[ts(jj, 128), :], in_=ct)
                    continue

                # ---- Wo + residual + LN1 ----
                r1 = mid.tile([128, CH, S], F32R, name="r1")
                q32 = mid.tile([128, CH, S], F32, name="q32")
                QDT = F8 if ffn_mode == "mix" else BF16
                q8 = mid.tile([128, CH, S], QDT, name="q8")
                with ExitStack() as s_c:
                    psC = s_c.enter_context(
                        tc.tile_pool(name="psC", bufs=1, space="PSUM"))
                    ln1 = LN(psC)
                    for jj in range(CH):
                        ps = psC.tile([128, S], F32, tag="mw", bufs=5,
                                      name="ps_wo")
                        for i in range(0, CH, 2):
                            nc.tensor.matmul(
                                ps[:, :], wo_sb[:, jj, i:i + 2, :],
                                ctx8[:, i:i + 2, :],
                                start=(i == 0), stop=(i == CH - 2),
                                perf_mode=PM.DoubleRow)
                        nc.vector.scalar_tensor_tensor(
                            out=r1[:, jj, :], in0=ps[:, :],
                            scalar=1.0 / (SW * SW), in1=xT[:, jj, :],
                            op0=ALU.mult, op1=ALU.add)
                        if jj >= 1:
                            ln1.accum(r1[:, jj - 1, :], jj - 1)
                    ln1.accum(r1[:, CH - 1, :], CH - 1)

                    def emit_ln1(jj, cen, rstd):
                        nc.vector.scalar_tensor_tensor(
                            out=q32[:, jj, :], in0=cen,
                            scalar=c_sb[:, L1G + jj:L1G + jj + 1], in1=rstd,
                            op0=ALU.mult, op1=ALU.mult)
                        nc.scalar.activation(out=q8[:, jj, :],
                                             in_=q32[:, jj, :],
                                             func=AF.Identity)

                    ln1.finish(r1, emit_ln1)
                    dump("d_r1", r1)
                    dump("d_q32", q32)
                    dump("d_q8", q8)

            if upto == "ln1":
                for jj in range(CH):
                    nc.sync.dma_start(out=out_d[ts(jj, 128), :],
                                      in_=q32[:, jj, :])
                continue

            # ---- FFN + LN2 ----
            with ExitStack() as s_de:
                gel8 = fpool.tile([128, CF, S], BF16, tag="gel8", bufs=1,
                                  name="gel8")
                psF = s_de.enter_context(
                    tc.tile_pool(name="psF", bufs=1, space="PSUM"))
                for fg in range(CF // 4):
                    w1t = w1pool.tile([128, 4, CH, 128],
                                      F8 if ffn_mode == "mix" else BF16,
                                      tag="w1", name="w1t")
                    nc.sync.dma_start(
                        out=w1t,
                        in_=w1_d.rearrange("f p i m -> p f i m")[
                            :, 4 * fg:4 * fg + 4, :, :])
                    for ff in range(4):
                        f = 4 * fg + ff
                        ps = psF.tile([128, S], F32, tag="m1", bufs=3,
                                      name="ps_f1")
                        if ffn_mode == "mix":
                            for i in range(0, CH, 2):
                                nc.tensor.matmul(
                                    ps[:, :], w1t[:, ff, i:i + 2, :],
                                    q8[:, i:i + 2, :],
                                    start=(i == 0), stop=(i == CH - 2),
                                    perf_mode=PM.DoubleRow)
                        else:
                            for i in range(CH):
                                nc.tensor.matmul(
                                    ps[:, :], w1t[:, ff, i, :],
                                    q8[:, i, :],
                                    start=(i == 0), stop=(i == CH - 1))
                        nc.scalar.activation(
                            out=gel8[:, f, :], in_=ps[:, :], func=AF.Gelu,
                            scale=ISW if ffn_mode == "mix" else 1.0,
                            bias=c_sb[:, B1E + f:B1E + f + 1])

                # previous rep's deferred LN2 chain: runs here because FFN1
                # keeps the PE busy while the DVE/Pool queues are idle
                if prev_fin is not None:
                    prev_fin()
                    prev_fin = None
                if prev_emit is not None:
                    for jd in range(CH):
                        prev_emit(jd)
                    prev_emit = None

                r2 = mid.tile([128, CH, S], F32R, name="r2")
                ln2 = LN(psF)

                def _ffn2_chunk(psF, w2t, jq, jj, r2):
                    ps = psF.tile([128, S], F32, tag="m2", bufs=3,
                                  name="ps_f2")
                    for i in range(CF):
                        nc.tensor.matmul(
                            ps[:, :], w2t[:, jq, i, :], gel8[:, i, :],
                            start=(i == 0), stop=(i == CF - 1))
                    # r2 = ffn2 + c2 + q32   (c2 = b2 + ln1_b, the LN2 shift)
                    nc.vector.scalar_tensor_tensor(
                        out=r2[:, jj, :], in0=ps[:, :],
                        scalar=c_sb[:, C2 + jj:C2 + jj + 1],
                        in1=q32[:, jj, :], op0=ALU.add, op1=ALU.add)

                for jg in range(CH // 2):
                    w2t = w2pool.tile([128, 2, CF, 128], BF16, tag="w2",
                                      name="w2t")
                    nc.sync.dma_start(
                        out=w2t,
                        in_=w2_d.rearrange("j p i m -> p j i m")[
                            :, 2 * jg:2 * jg + 2, :, :])
                    for jq in range(2):
                        jj = 2 * jg + jq
                        _ffn2_chunk(psF, w2t, jq, jj, r2)
                        if jj >= 1:
                            ln2.accum(r2[:, jj - 1, :], jj - 1)
                ln2.accum(r2[:, CH - 1, :], CH - 1)

                dump("d_gel", gel8)
                dump("d_r2", r2)

                def emit_ln2(jj, cen, rstd):
                    # cen alternates Pool/DVE (see layer_norm); stagger the
                    # rest of the chain across DVE/Pool/ACT by chunk parity
                    # so the end-of-layer tail pipelines across engines.
                    # this chain runs overlapped with the next rep's
                    # attention: keep it off ACT (exp-saturated there)
                    nrm = tmp.tile([128, S], F32, tag="nrm2", bufs=2,
                                   name="nrm2")
                    eng = nc.vector if jj % 2 == 0 else nc.gpsimd
                    eng.tensor_tensor(out=nrm, in0=cen, in1=rstd,
                                      op=ALU.mult)
                    ot = tmp.tile([128, S], F32, tag="ot", bufs=2, name="ot")
                    nc.vector.tensor_scalar(
                        out=ot, in0=nrm,
                        scalar1=c_sb[:, L2G + jj:L2G + jj + 1],
                        scalar2=c_sb[:, L2B + jj:L2B + jj + 1],
                        op0=ALU.mult, op1=ALU.add)
                    nc.sync.dma_start(out=out_d[ts(jj, 128), :], in_=ot)

                ln2.finish_stats()
                prev_fin = ln2.defer_stats
                prev_emit = (lambda j, ln=ln2, r=r2, e=emit_ln2:
                             ln.emit_chunk(r, j, e))

        # drain the last rep's deferred LN2
        if prev_fin is not None:
            prev_fin()
        if prev_emit is not None:
            for j in range(CH):
                prev_emit(j)

    nc.finalize()
    return nc


_NC_CACHE = None


def _get_nc():
    global _NC_CACHE
    if _NC_CACHE is None:
        _NC_CACHE = build_nc()
    return _NC_CACHE


def make_in_maps(hidden_states, attention_mask, Wq, bq, Wk, bk, Wv, bv, Wo, bo,
                 ln1_g, ln1_b, W1, b1, W2, b2, ln2_g, ln2_b):
    """Host-side sharding + layout prep. Returns one input map per core."""
    f32 = np.float32
    f8 = ml_dtypes.float8_e4m3fn
    bf16np = ml_dtypes.bfloat16
    Wq, Wk, Wv, Wo = (np.asarray(w, f32) for w in (Wq, Wk, Wv, Wo))
    W1, W2 = np.asarray(W1, f32), np.asarray(W2, f32)
    bo_eff = np.asarray(bo, f32) + np.asarray(bv, f32) @ Wo
    beta1 = np.asarray(ln1_b, f32)
    b1_eff = np.asarray(b1, f32) + beta1 @ W1
    c2 = np.asarray(b2, f32) + beta1

    def blocks(w, co, ci):
        # [ci*128, co*128] -> [co, 128(k), ci, 128(m)] fp8, scaled
        return np.ascontiguousarray(
            (w * SW).reshape(ci, 128, co, 128).transpose(2, 1, 0, 3)
        ).astype(f8)

    def blocks_bf16(w, co, ci):
        return np.ascontiguousarray(
            w.reshape(ci, 128, co, 128).transpose(2, 1, 0, 3)
        ).astype(bf16np)

    wqkb = np.ascontiguousarray(
        np.stack([blocks(Wq, CH, CH), blocks(Wk, CH, CH)], axis=2))
    wob = blocks(Wo, CH, CH)
    # permute Wv output features: even heads first, then odd heads, so each
    # V-projection half covers one parity (matches v8m's parity layout)
    vperm = np.concatenate(
        [np.arange(h * DH, (h + 1) * DH)
         for h in list(range(0, NH, 2)) + list(range(1, NH, 2))])
    Wv = Wv[:, vperm]
    if FFN_MODE == "mix":
        w1b = blocks(W1, CF, CH)
    else:
        w1b = blocks_bf16(W1, CF, CH)
    w2b = blocks_bf16(W2, CH, CF)
    wv8 = (Wv * SW).astype(f8)

    def cols(v, n):
        return np.ascontiguousarray(np.asarray(v, f32).reshape(n, 128).T)

    base = np.zeros((128, NCONST), f32)
    base[:, BQ:BQ + CH] = cols(bq, CH)
    base[:, BK:BK + CH] = cols(bk, CH)
    base[:, L1G:L1G + CH] = cols(ln1_g, CH)
    base[:, C2:C2 + CH] = cols(c2, CH)
    base[:, L2G:L2G + CH] = cols(ln2_g, CH)
    base[:, L2B:L2B + CH] = cols(ln2_b, CH)
    base[:, B1E:B1E + CF] = cols(b1_eff, CF)

    mask = np.asarray(attention_mask, f32)  # [B, S]
    x = np.asarray(hidden_states, f32)
    in_maps = []
    for b in range(B):
        consts = base.copy()
        consts[:, MCOL:MCOL + T] = cols(mask[b] * ISW, T)
        xT = np.ascontiguousarray(x[b].T)
        in_maps.append({
            "xT": np.ascontiguousarray(xT + bo_eff[:, None]),
            "xT8": xT.astype(f8),
            "wqkb": wqkb, "wv": wv8, "wob": wob,
            "w1b": w1b, "w2b": w2b,
            "consts": consts,
        })
    return in_maps


def kernel(**inputs):
    nc = _get_nc()
    in_maps = make_in_maps(**inputs)
    res = run_bass_kernel_spmd(nc, in_maps, core_ids=list(range(B)))
    out = np.stack([np.ascontiguousarray(r["outT"].T) for r in res.results])
    return out.astype(np.float32)


# revision 61
# speedup vs baseline: 1.1957x; 1.1957x over previous
"""BERT layer (B=8, S=512, H=768, NH=12, DH=64, FF=3072) on 8 Trainium2 cores.

Data-parallel over batch (1 element/core).  Feature-major on-chip layout
(activations as X^T [H partitions, S free]).  All contraction>=256 matmuls run
fp8e4m3 DoubleRow (2x PE rate): QKV projections, ctx, Wo, FFN1.  FFN2 stays
bf16 for accuracy.  Weights are host-scaled x64 so fp8 stays in normal range;
the 1/64 unscale is folded into the PSUM-evacuation ops that exist anyway.
Scores stay bf16 (K=64, row-packed head pairs); LayerNorm stats stay f32r.

Softmax: the additive 0/1 mask is folded multiplicatively (exp(s+mb)=exp(s)*m):
V is scaled by m/64 on evacuation, and the softmax denominator comes FOR FREE
out of the ctx matmul: the ctx stationary operand is widened to 128 columns
[V_head | mask/64 replicated], so PSUM rows 0-63 hold the ctx numerator and
rows 64-127 hold the denominator (replicated).  recip = 64/den cancels the
1/64 and lands ctx at x64 scale, the fp8-friendly range for the Wo input.

LayerNorm inputs are pre-shifted so no bias plumbing is needed in the stats:
  LN1: xT carries x^T + bo_eff (bo_eff = bo + bv@Wo) from the host.
  LN2: r2 = ffn2 + c2 + q32 in one scalar_tensor_tensor (c2 = b2 + ln1_b).
rstd = exp(-0.5*ln(var+eps)) keeps the ACT table set at
natural_log_exp_and_others (shared with softmax exp): only 2 table switches
per layer (to/from the Gelu set).

Engine balance: ACT keeps only table ops (exp/gelu/ln-exp); squares, copies,
and the final scale+bias run on DVE; cen/mean^2/q32/nrm run on Pool (gpsimd),
which is otherwise idle.  FFN weight DMAs issue from the sync queue.

ln1_b (beta1) folding (exact): h1 = gamma1*nrm + beta1.  The fp8 h1 fed to
FFN1 omits beta1 (compensated by b1_eff = b1 + beta1@W1); the residual
carries q = gamma1*nrm exactly, with beta1 folded into LN2's shift
c2 = b2 + beta1.
"""

from contextlib import ExitStack

import numpy as np
import ml_dtypes

from concourse import bacc
import concourse.tile as tile
from concourse import mybir
from concourse.bass_utils import run_bass_kernel_spmd

F32 = mybir.dt.float32
F32R = mybir.dt.float32r
BF16 = mybir.dt.bfloat16
F8 = mybir.dt.float8e4
AF = mybir.ActivationFunctionType
ALU = mybir.AluOpType
PM = mybir.MatmulPerfMode

B, S, H, NH, DH, FF = 8, 512, 768, 12, 64, 3072
CH = H // 128   # 6 hidden chunks
CF = FF // 128  # 24 ff chunks
T = S // 128    # 4 token/key chunks
NP = NH // 2    # 6 head pairs
EPS = 1e-3
FFN_MODE = "mix"  # "mix" (ff1 fp8 + ff2 bf16) | "bf16"
K8 = 12         # FFN2 contraction chunks done in fp8 DoubleRow (of CF=24)
SW = 64.0       # weight scale (keeps fp8 in normal range)
ISW = 1.0 / SW

# consts tile column map: [128, NCONST]
BQ, BK, L1G, C2, L2G, L2B = 0, 6, 12, 18, 24, 30
MCOL = 36        # 4 cols: mask/64 per key chunk
B1E = 40         # 24 cols: b1 + ln1_b @ W1
NCONST = B1E + CF


def ts(i, n):
    return slice(i * n, (i + 1) * n)


def build_nc(repeats=1, ffn_mode=None, dbg=False, upto="full"):
    ffn_mode = ffn_mode or FFN_MODE
    nc = bacc.Bacc("TRN2", target_bir_lowering=False, debug=False)
    dbg_d = {}
    if dbg:
        for nm, shp, dt_ in (
                ("d_qT", [128, CH, S], BF16), ("d_kT", [128, CH, S], BF16),
                ("d_v8m", [128, T, NP, 2, 128], F8), ("d_es0", [128, T, S], F8),
                ("d_ctx8", [128, CH, S], F8), ("d_r1", [128, CH, S], F32R),
                ("d_q32", [128, CH, S], F32), ("d_q8", [128, CH, S], F8),
                ("d_gel", [128, CF - K8, S], BF16), ("d_r2", [128, CH, S], F32R)):
            dbg_d[nm] = nc.declare_dram_parameter(nm, shp, dt_, isOutput=True)

    def dump(nm, t):
        if dbg:
            nc.sync.dma_start(out=dbg_d[nm][...], in_=t)

    xT_d = nc.declare_dram_parameter("xT", [H, S], F32R, isOutput=False)
    xT8_d = nc.declare_dram_parameter("xT8", [H, S], F8, isOutput=False)
    wqk_d = nc.declare_dram_parameter("wqkb", [CH, 128, 2, CH, 128], F8,
                                      isOutput=False)
    wv_d = nc.declare_dram_parameter("wv", [H, H], F8, isOutput=False)
    wob_d = nc.declare_dram_parameter("wob", [CH, 128, CH, 128], F8,
                                      isOutput=False)
    W1DT = F8 if ffn_mode == "mix" else BF16
    w1_d = nc.declare_dram_parameter("w1b", [CF, 128, CH, 128], W1DT,
                                     isOutput=False)
    w2a_d = nc.declare_dram_parameter("w2a", [CH, 128, K8, 128], F8,
                                      isOutput=False)
    w2_d = nc.declare_dram_parameter("w2b", [CH, 128, CF - K8, 128], BF16,
                                     isOutput=False)
    c_d = nc.declare_dram_parameter("consts", [128, NCONST], F32,
                                    isOutput=False)
    out_d = nc.declare_dram_parameter("outT", [H, S], F32, isOutput=True)

    def fmaj(d):
        return d.rearrange("(i p) n -> p i n", p=128)

    with tile.TileContext(nc) as tc, ExitStack() as top:
        cpool = top.enter_context(tc.tile_pool(name="cpool", bufs=1))
        c_sb = cpool.tile([128, NCONST], F32, name="c_sb")
        nc.sync.dma_start(out=c_sb, in_=c_d[:, :])
        ones_f32 = cpool.tile([128, 384], F32, name="ones_f32")
        nc.vector.memset(ones_f32, 1.0)
        ones_sum = cpool.tile([128, 128], F32R, name="ones_sum")
        nc.vector.tensor_copy(out=ones_sum, in_=ones_f32[:, 0:128])

        mid = top.enter_context(tc.tile_pool(name="mid", bufs=1))
        tmp = top.enter_context(tc.tile_pool(name="tmp", bufs=1))
        fpool = top.enter_context(tc.tile_pool(name="fpool", bufs=1))
        w1pool = top.enter_context(tc.tile_pool(name="w1p", bufs=3))
        w2pool = top.enter_context(tc.tile_pool(name="w2p", bufs=2))

        class LN:
            """Incremental LayerNorm over pre-shifted feature-major src.

            accum(i) is called as each src chunk becomes ready so the
            sum/sq matmuls interleave with the producing loop; the sq
            matmuls trail two chunks so their DVE/Pool square op has time
            to complete before the PE reaches the matmul.
            """

            def __init__(self, pssum):
                self.sum_ps = pssum.tile([128, S], F32, tag="lnsum", bufs=1,
                                         name="sum_ps")
                self.sq_ps = pssum.tile([128, S], F32, tag="lnsq", bufs=1,
                                        name="sq_ps")
                self.pending = []
                self.k = 0

            def accum(self, src_i, i):
                nc.tensor.matmul(self.sum_ps[:, :], ones_sum[:, :], src_i,
                                 start=(i == 0), stop=(i == CH - 1))
                # Square on ACT: in every table set (no table-load cost) and
                # ACT is idle in both LN windows
                sq = tmp.tile([128, S], F32R, tag="sq", bufs=3, name="sq")
                nc.scalar.activation(out=sq, in_=src_i, func=AF.Square)
                self.pending.append(sq)
                while len(self.pending) > 2:
                    self._sq_mm()

            def _sq_mm(self):
                sq = self.pending.pop(0)
                nc.tensor.matmul(self.sq_ps[:, :], ones_sum[:, :], sq,
                                 start=(self.k == 0), stop=(self.k == CH - 1))
                self.k += 1

            def finish_stats(self):
                """Evacuate the PSUM sums (the only PSUM-dependent step)."""
                while self.pending:
                    self._sq_mm()
                self.mean = tmp.tile([128, S], F32, tag="mean", bufs=2,
                                     name="mean")
                nc.scalar.activation(out=self.mean, in_=self.sum_ps[:, :],
                                     func=AF.Identity, scale=1.0 / H)
                self.ve = tmp.tile([128, S], F32, tag="ve", bufs=2,
                                   name="ve")
                nc.vector.tensor_scalar(
                    out=self.ve, in0=self.sq_ps[:, :], scalar1=1.0 / H,
                    scalar2=EPS, op0=ALU.mult, op1=ALU.add)

            def defer_stats(self):
                """SBUF-only stats chain; safe to run any time after
                finish_stats, e.g. overlapped with the next rep."""
                m2 = tmp.tile([128, S], F32, tag="m2", bufs=2, name="m2")
                nc.gpsimd.tensor_tensor(out=m2, in0=self.mean, in1=self.mean,
                                        op=ALU.mult)
                var = tmp.tile([128, S], F32, tag="var", bufs=2, name="var")
                nc.vector.tensor_tensor(out=var, in0=self.ve, in1=m2,
                                        op=ALU.subtract)
                std = tmp.tile([128, S], F32, tag="std", bufs=2, name="std")
                nc.scalar.activation(out=std, in_=var, func=AF.Sqrt)
                self.rstd = tmp.tile([128, S], F32, tag="rstd", bufs=2,
                                     name="rstd")
                nc.vector.reciprocal_approx_fast(out=self.rstd, in_=std)

            def emit_chunk(self, src, jj, emit, spread=False):
                cen = tmp.tile([128, S], F32, tag="cen", bufs=2,
                               name="cen")
                eng = nc.gpsimd if (spread and jj % 2 == 0) else nc.vector
                eng.tensor_tensor(out=cen, in0=src[:, jj, :], in1=self.mean,
                                  op=ALU.subtract)
                emit(jj, cen, self.rstd)

            def finish(self, src, emit):
                self.finish_stats()
                self.defer_stats()
                for jj in range(CH):
                    self.emit_chunk(src, jj, emit)

        # deferred LN2 of the previous rep: SBUF-only ops overlapped with the
        # next rep's attention phase (which has PE/exp as its bottleneck)
        prev_fin = None
        prev_emit = None

        for _rep in range(repeats):
            with ExitStack() as s_ac:
                apool = s_ac.enter_context(tc.tile_pool(name="apool", bufs=1))
                xT = apool.tile([128, CH, S], F32R, tag="xT", bufs=1, name="xT")
                xT8 = apool.tile([128, CH, S], F8, tag="xT8", bufs=1, name="xT8")
                nc.sync.dma_start(out=xT8, in_=fmaj(xT8_d))
                # xT (residual) is first read at the Wo evacuation; issue it
                # on the idle Pool queue (SWDGE) so it doesn't delay the
                # early-phase sync-queue inputs
                nc.gpsimd.dma_start(out=xT, in_=fmaj(xT_d))
                qT = apool.tile([128, CH, S], BF16, tag="qT", bufs=1, name="qT")
                kT = apool.tile([128, CH, S], BF16, tag="kT", bufs=1, name="kT")
                # v8m [p, t, pair, half, 128]: per head 128 stationary cols.
                # Even heads (half 0): [mask/64 | V]; odd heads: [V | mask/64].
                # The parity-dependent layout keeps the reciprocal's in/out
                # partition bases equal (custom-DVE op requires it).
                v8m = apool.tile([128, T, NP, 2, 128], F8, tag="v8m", bufs=2,
                                 name="v8m")
                ctx8 = apool.tile([128, CH, S], F8, tag="ctx8", bufs=1, name="ctx8")

                wpool = s_ac.enter_context(tc.tile_pool(name="wpool", bufs=1))
                wo_sb = wpool.tile([128, CH, CH, 128], F8, name="wo_sb")

                psP = s_ac.enter_context(
                    tc.tile_pool(name="psP", bufs=1, space="PSUM"))
                wqkpool = s_ac.enter_context(tc.tile_pool(name="wqk", bufs=3))

                def project_qk(j):
                    wt = wqkpool.tile([128, 2, CH, 128], F8, tag="wqk",
                                      name="wt")
                    nc.sync.dma_start(out=wt, in_=wqk_d[j])
                    for ci, (dest, bcol) in enumerate(((qT, BQ), (kT, BK))):
                        ps = psP.tile([128, S], F32, tag="pj", bufs=1,
                                      name="ps_qk")
                        for i in range(0, CH, 2):
                            nc.tensor.matmul(
                                ps[:, :], wt[:, ci, i:i + 2, :],
                                xT8[:, i:i + 2, :],
                                start=(i == 0), stop=(i == CH - 2),
                                perf_mode=PM.DoubleRow)
                        nc.vector.tensor_scalar(
                            out=dest[:, j, :], in0=ps[:, :], scalar1=ISW,
                            scalar2=c_sb[:, bcol + j:bcol + j + 1],
                            op0=ALU.mult, op1=ALU.add)

                # mask/64 replica columns of v8m (per key chunk t)
                ones6 = ones_f32[:, 0:384].rearrange("p (h d) -> p h d", h=NP)
                for t in range(T):
                    for half in range(2):
                        nc.vector.tensor_scalar(
                            out=v8m[:, t, :, half, ts(half, 64)],
                            in0=ones6,
                            scalar1=c_sb[:, MCOL + t:MCOL + t + 1],
                            scalar2=None, op0=ALU.mult)

                # ---- V projection (token-major), then Q/K of pair 0 ----
                with ExitStack() as s_v:
                    wvpool = s_v.enter_context(
                        tc.tile_pool(name="wvp", bufs=1))
                    wv_sb = wvpool.tile([128, CH, H], F8, name="wv_sb")
                    nc.sync.dma_start(out=wv_sb, in_=fmaj(wv_d))
                    psV = s_v.enter_context(
                        tc.tile_pool(name="psV", bufs=1, space="PSUM"))
                    for t in range(T):
                        for half in range(2):
                            ps = psV.tile([128, 512], F32, tag="mv", bufs=4,
                                          name="ps_v")
                            for i in range(0, CH, 2):
                                nc.tensor.matmul(
                                    ps[:, 0:384],
                                    xT8[:, i:i + 2, ts(t, 128)],
                                    wv_sb[:, i:i + 2, ts(half, 384)],
                                    start=(i == 0), stop=(i == CH - 2),
                                    perf_mode=PM.DoubleRow)
                            # wv cols are host-permuted by head parity:
                            # vhalf 0 = even heads, vhalf 1 = odd heads
                            nc.vector.tensor_scalar(
                                out=v8m[:, t, :, half, ts(1 - half, 64)],
                                in0=ps[:, 0:384].rearrange(
                                    "p (h d) -> p h d", h=6),
                                scalar1=c_sb[:, MCOL + t:MCOL + t + 1],
                                scalar2=None, op0=ALU.mult)
                    project_qk(0)

                # ---- pair loop: scores -> exp -> ctx+den ----
                with ExitStack() as s_b:
                    psS = s_b.enter_context(
                        tc.tile_pool(name="psS", bufs=1, space="PSUM"))
                    psD = s_b.enter_context(
                        tc.tile_pool(name="psD", bufs=1, space="PSUM"))
                    bpool = s_b.enter_context(
                        tc.tile_pool(name="bpool", bufs=1))
                    es_store = {}

                    def do_scores(j):
                        # interleave the two head-halves' matmuls so
                        # consecutive MMs hit different PE row-groups and
                        # overlap in the array
                        es_ab = [
                            bpool.tile([128, T, S], F8, tag=f"es{half}",
                                       bufs=3, name=f"es{half}")
                            for half in range(2)
                        ]
                        for u in range(2):
                            scs = [psS.tile([128, 1024], F32, tag="sc",
                                            bufs=2, name="sc_ps")
                                   for _ in range(2)]
                            for v_ in range(2):
                                t = 2 * u + v_
                                for half in range(2):
                                    nc.tensor.matmul(
                                        scs[half][:, ts(v_, 512)],
                                        kT[ts(half, 64), j, ts(t, 128)],
                                        qT[ts(half, 64), j, :],
                                        start=True, stop=True,
                                        tile_position=(half * 64, 0))
                            for half in range(2):
                                nc.scalar.activation(
                                    out=es_ab[half][
                                        :, 2 * u:2 * u + 2, :].rearrange(
                                        "p a b -> p (a b)"),
                                    in_=scs[half][:, :], func=AF.Exp,
                                    scale=0.125)
                        es_store[j] = es_ab

                    def do_ctx(j):
                        # ctx+den: DoubleRow per head; stationary 128 cols =
                        # [mask/64 | V] (even heads) / [V | mask/64] (odd), so
                        # one PSUM half is the numerator and the other the
                        # replicated denominator.  The reciprocal (custom-DVE
                        # microcode) needs full base-0 tiles, so it runs over
                        # all 128 rows; non-den rows are garbage, never read.
                        # The parity keeps the multiply's SBUF in1/out
                        # partition bases equal.
                        es_ab = es_store.pop(j)
                        for half, es in enumerate(es_ab):
                            ctx_ps = psD.tile([128, S], F32, tag="ctx",
                                              bufs=2, name="ctx_ps")
                            for u in range(2):
                                nc.tensor.matmul(
                                    ctx_ps[:, :],
                                    v8m[:, 2 * u:2 * u + 2, j, half, :],
                                    es[:, 2 * u:2 * u + 2, :],
                                    start=(u == 0), stop=(u == 1),
                                    perf_mode=PM.DoubleRow)
                            recip = bpool.tile([128, S], F32, tag="recip",
                                               bufs=2, name="recip")
                            nc.vector.reciprocal_approx_fast(
                                out=recip, in_=ctx_ps[:, :])
                            nc.vector.tensor_tensor(
                                out=ctx8[ts(half, 64), j, :],
                                in0=ctx_ps[ts(1 - half, 64), :],
                                in1=recip[ts(half, 64), :], op=ALU.mult)

                    # software pipeline: ctx runs one pair behind scores so
                    # the in-order PE queue never waits on the exp (ACT) of
                    # the current pair.
                    for j in range(NP):
                        if j > 0:
                            project_qk(j)
                        if j == 2:
                            nc.sync.dma_start(
                                out=wo_sb,
                                in_=wob_d.rearrange("j p i m -> p j i m"))
                        do_scores(j)
                        if j > 0:
                            do_ctx(j - 1)
                    do_ctx(NP - 1)

                dump("d_qT", qT)
                dump("d_kT", kT)
                dump("d_v8m", v8m)
                dump("d_ctx8", ctx8)

                if upto == "attn":
                    for jj in range(CH):
                        ct = tmp.tile([128, S], F32, tag="ct", bufs=2,
                                      name="ct")
                        nc.vector.tensor_copy(out=ct, in_=ctx8[:, jj, :])
                        nc.sync.dma_start(out=out_d[ts(jj, 128), :], in_=ct)
                    continue

                # ---- Wo + residual + LN1 ----
                r1 = mid.tile([128, CH, S], F32R, name="r1")
                q32 = mid.tile([128, CH, S], F32, name="q32")
                QDT = F8 if ffn_mode == "mix" else BF16
                q8 = mid.tile([128, CH, S], QDT, name="q8")
                with ExitStack() as s_c:
                    psC = s_c.enter_context(
                        tc.tile_pool(name="psC", bufs=1, space="PSUM"))
                    ln1 = LN(psC)
                    for jj in range(CH):
                        ps = psC.tile([128, S], F32, tag="mw", bufs=5,
                                      name="ps_wo")
                        for i in range(0, CH, 2):
                            nc.tensor.matmul(
                                ps[:, :], wo_sb[:, jj, i:i + 2, :],
                                ctx8[:, i:i + 2, :],
                                start=(i == 0), stop=(i == CH - 2),
                                perf_mode=PM.DoubleRow)
                        nc.vector.scalar_tensor_tensor(
                            out=r1[:, jj, :], in0=ps[:, :],
                            scalar=1.0 / (SW * SW), in1=xT[:, jj, :],
                            op0=ALU.mult, op1=ALU.add)
                        if jj >= 1:
                            ln1.accum(r1[:, jj - 1, :], jj - 1)
                    ln1.accum(r1[:, CH - 1, :], CH - 1)

                    def emit_ln1(jj, cen, rstd):
                        nc.vector.scalar_tensor_tensor(
                            out=q32[:, jj, :], in0=cen,
                            scalar=c_sb[:, L1G + jj:L1G + jj + 1], in1=rstd,
                            op0=ALU.mult, op1=ALU.mult)
                        nc.scalar.activation(out=q8[:, jj, :],
                                             in_=q32[:, jj, :],
                                             func=AF.Identity)

                    ln1.finish(r1, emit_ln1)
                    dump("d_r1", r1)
                    dump("d_q32", q32)
                    dump("d_q8", q8)

            if upto == "ln1":
                for jj in range(CH):
                    nc.sync.dma_start(out=out_d[ts(jj, 128), :],
                                      in_=q32[:, jj, :])
                continue

            # ---- FFN + LN2 ----
            with ExitStack() as s_de:
                # gelu output: first K8 chunks in fp8 (natural scale — gelu
                # range fits e4m3), rest bf16, for the split-precision FFN2
                gel8a = fpool.tile([128, K8, S], F8, tag="gel8a", bufs=1,
                                   name="gel8a")
                gel8 = fpool.tile([128, CF - K8, S], BF16, tag="gel8", bufs=1,
                                  name="gel8")
                psF = s_de.enter_context(
                    tc.tile_pool(name="psF", bufs=1, space="PSUM"))
                for fg in range(CF // 4):
                    w1t = w1pool.tile([128, 4, CH, 128],
                                      F8 if ffn_mode == "mix" else BF16,
                                      tag="w1", name="w1t")
                    nc.sync.dma_start(
                        out=w1t,
                        in_=w1_d.rearrange("f p i m -> p f i m")[
                            :, 4 * fg:4 * fg + 4, :, :])
                    for ff in range(4):
                        f = 4 * fg + ff
                        ps = psF.tile([128, S], F32, tag="m1", bufs=2,
                                      name="ps_f1")
                        if ffn_mode == "mix":
                            for i in range(0, CH, 2):
                                nc.tensor.matmul(
                                    ps[:, :], w1t[:, ff, i:i + 2, :],
                                    q8[:, i:i + 2, :],
                                    start=(i == 0), stop=(i == CH - 2),
                                    perf_mode=PM.DoubleRow)
                        else:
                            for i in range(CH):
                                nc.tensor.matmul(
                                    ps[:, :], w1t[:, ff, i, :],
                                    q8[:, i, :],
                                    start=(i == 0), stop=(i == CH - 1))
                        gdst = (gel8a[:, f, :] if f < K8
                                else gel8[:, f - K8, :])
                        nc.scalar.activation(
                            out=gdst, in_=ps[:, :], func=AF.Gelu,
                            scale=ISW if ffn_mode == "mix" else 1.0,
                            bias=c_sb[:, B1E + f:B1E + f + 1])

                # previous rep's deferred LN2 chain: runs here because FFN1
                # keeps the PE busy while the DVE/Pool queues are idle
                if prev_fin is not None:
                    prev_fin()
                    prev_fin = None
                if prev_emit is not None:
                    for jd in range(CH):
                        prev_emit(jd)
                    prev_emit = None

                r2 = mid.tile([128, CH, S], F32R, name="r2")
                ln2 = LN(psF)

                def _ffn2_chunk(psF, w2at, w2t, jq, jj, r2):
                    # fp8 DoubleRow part: product scale x64 (W2 host-scaled),
                    # separate PSUM accumulator from the bf16 part
                    psa = psF.tile([128, S], F32, tag="m2a", bufs=2,
                                   name="ps_f2a")
                    for i in range(0, K8, 2):
                        nc.tensor.matmul(
                            psa[:, :], w2at[:, jq, i:i + 2, :],
                            gel8a[:, i:i + 2, :],
                            start=(i == 0), stop=(i == K8 - 2),
                            perf_mode=PM.DoubleRow)
                    ps = psF.tile([128, S], F32, tag="m2", bufs=2,
                                  name="ps_f2")
                    for i in range(CF - K8):
                        nc.tensor.matmul(
                            ps[:, :], w2t[:, jq, i, :], gel8[:, i, :],
                            start=(i == 0), stop=(i == CF - K8 - 1))
                    # r2 = ffn2_fp8/64 + ffn2_bf16 + c2 + q32
                    f2t = tmp.tile([128, S], F32, tag="f2t", bufs=2,
                                   name="f2t")
                    nc.vector.scalar_tensor_tensor(
                        out=f2t, in0=psa[:, :], scalar=ISW,
                        in1=q32[:, jj, :], op0=ALU.mult, op1=ALU.add)
                    nc.vector.scalar_tensor_tensor(
                        out=r2[:, jj, :], in0=ps[:, :],
                        scalar=c_sb[:, C2 + jj:C2 + jj + 1],
                        in1=f2t, op0=ALU.add, op1=ALU.add)

                for jg in range(CH // 2):
                    w2at = w2pool.tile([128, 2, K8, 128], F8, tag="w2a",
                                       name="w2at")
                    nc.sync.dma_start(
                        out=w2at,
                        in_=w2a_d.rearrange("j p i m -> p j i m")[
                            :, 2 * jg:2 * jg + 2, :, :])
                    w2t = w2pool.tile([128, 2, CF - K8, 128], BF16, tag="w2",
                                      name="w2t")
                    nc.sync.dma_start(
                        out=w2t,
                        in_=w2_d.rearrange("j p i m -> p j i m")[
                            :, 2 * jg:2 * jg + 2, :, :])
                    for jq in range(2):
                        jj = 2 * jg + jq
                        _ffn2_chunk(psF, w2at, w2t, jq, jj, r2)
                        if jj >= 1:
                            ln2.accum(r2[:, jj - 1, :], jj - 1)
                ln2.accum(r2[:, CH - 1, :], CH - 1)

                dump("d_gel", gel8)
                dump("d_r2", r2)

                def emit_ln2(jj, cen, rstd):
                    # cen alternates Pool/DVE (see layer_norm); stagger the
                    # rest of the chain across DVE/Pool/ACT by chunk parity
                    # so the end-of-layer tail pipelines across engines.
                    # this chain runs overlapped with the next rep's
                    # attention: keep it off ACT (exp-saturated there)
                    nrm = tmp.tile([128, S], F32, tag="nrm2", bufs=2,
                                   name="nrm2")
                    eng = nc.vector if jj % 2 == 0 else nc.gpsimd
                    eng.tensor_tensor(out=nrm, in0=cen, in1=rstd,
                                      op=ALU.mult)
                    ot = tmp.tile([128, S], F32, tag="ot", bufs=2, name="ot")
                    nc.vector.tensor_scalar(
                        out=ot, in0=nrm,
                        scalar1=c_sb[:, L2G + jj:L2G + jj + 1],
                        scalar2=c_sb[:, L2B + jj:L2B + jj + 1],
                        op0=ALU.mult, op1=ALU.add)
                    nc.sync.dma_start(out=out_d[ts(jj, 128), :], in_=ot)

                ln2.finish_stats()
                prev_fin = ln2.defer_stats
                prev_emit = (lambda j, ln=ln2, r=r2, e=emit_ln2:
                             ln.emit_chunk(r, j, e))

        # drain the last rep's deferred LN2
        if prev_fin is not None:
            prev_fin()
        if prev_emit is not None:
            for j in range(CH):
                prev_emit(j)

    nc.finalize()
    return nc


_NC_CACHE = None


def _get_nc():
    global _NC_CACHE
    if _NC_CACHE is None:
        _NC_CACHE = build_nc()
    return _NC_CACHE


def make_in_maps(hidden_states, attention_mask, Wq, bq, Wk, bk, Wv, bv, Wo, bo,
                 ln1_g, ln1_b, W1, b1, W2, b2, ln2_g, ln2_b):
    """Host-side sharding + layout prep. Returns one input map per core."""
    f32 = np.float32
    f8 = ml_dtypes.float8_e4m3fn
    bf16np = ml_dtypes.bfloat16
    Wq, Wk, Wv, Wo = (np.asarray(w, f32) for w in (Wq, Wk, Wv, Wo))
    W1, W2 = np.asarray(W1, f32), np.asarray(W2, f32)
    bo_eff = np.asarray(bo, f32) + np.asarray(bv, f32) @ Wo
    beta1 = np.asarray(ln1_b, f32)
    b1_eff = np.asarray(b1, f32) + beta1 @ W1
    c2 = np.asarray(b2, f32) + beta1

    def blocks(w, co, ci):
        # [ci*128, co*128] -> [co, 128(k), ci, 128(m)] fp8, scaled
        return np.ascontiguousarray(
            (w * SW).reshape(ci, 128, co, 128).transpose(2, 1, 0, 3)
        ).astype(f8)

    def blocks_bf16(w, co, ci):
        return np.ascontiguousarray(
            w.reshape(ci, 128, co, 128).transpose(2, 1, 0, 3)
        ).astype(bf16np)

    wqkb = np.ascontiguousarray(
        np.stack([blocks(Wq, CH, CH), blocks(Wk, CH, CH)], axis=2))
    wob = blocks(Wo, CH, CH)
    # permute Wv output features: even heads first, then odd heads, so each
    # V-projection half covers one parity (matches v8m's parity layout)
    vperm = np.concatenate(
        [np.arange(h * DH, (h + 1) * DH)
         for h in list(range(0, NH, 2)) + list(range(1, NH, 2))])
    Wv = Wv[:, vperm]
    if FFN_MODE == "mix":
        w1b = blocks(W1, CF, CH)
    else:
        w1b = blocks_bf16(W1, CF, CH)
    w2a = blocks(W2[:K8 * 128], CH, K8)
    w2b = blocks_bf16(W2[K8 * 128:], CH, CF - K8)
    wv8 = (Wv * SW).astype(f8)

    def cols(v, n):
        return np.ascontiguousarray(np.asarray(v, f32).reshape(n, 128).T)

    base = np.zeros((128, NCONST), f32)
    base[:, BQ:BQ + CH] = cols(bq, CH)
    base[:, BK:BK + CH] = cols(bk, CH)
    base[:, L1G:L1G + CH] = cols(ln1_g, CH)
    base[:, C2:C2 + CH] = cols(c2, CH)
    base[:, L2G:L2G + CH] = cols(ln2_g, CH)
    base[:, L2B:L2B + CH] = cols(ln2_b, CH)
    base[:, B1E:B1E + CF] = cols(b1_eff, CF)

    mask = np.asarray(attention_mask, f32)  # [B, S]
    x = np.asarray(hidden_states, f32)
    in_maps = []
    for b in range(B):
        consts = base.copy()
        consts[:, MCOL:MCOL + T] = cols(mask[b] * ISW, T)
        xT = np.ascontiguousarray(x[b].T)
        in_maps.append({
            "xT": np.ascontiguousarray(xT + bo_eff[:, None]),
            "xT8": xT.astype(f8),
            "wqkb": wqkb, "wv": wv8, "wob": wob,
            "w1b": w1b, "w2a": w2a, "w2b": w2b,
            "consts": consts,
        })
    return in_maps


def kernel(**inputs):
    nc = _get_nc()
    in_maps = make_in_maps(**inputs)
    res = run_bass_kernel_spmd(nc, in_maps, core_ids=list(range(B)))
    out = np.stack([np.ascontiguousarray(r["outT"].T) for r in res.results])
    return out.astype(np.float32)
